# revision 81
# baseline (speedup 1.0000x reference)
"""Trainium2 Bass kernel for a dense pre-norm transformer block (v3).

Reference semantics (per batch b, seq len T=100, d_model D=384, heads H=6):
  h   = LN(x) * g1 + beta1
  q,k,v = per-head projections of h;  wei = softmax(mask(q k^T * sqrt(64)))
  x2  = x + (wei v) Wp + bp
  out = x2 + relu(LN(x2)*g2+beta2 @ W1 + b1) @ W2 + b2

Distribution: data-parallel over the batch dim across 8 NeuronCores
(64 batches each); weights are replicated. No collectives.

Numerics: proj/FFN1/FFN2 run fp8(e4m3) with MatmulPerfMode.DoubleRow
(k-tile pairs, packed [128, ..., 2, X] layouts); Q/K/V stay bf16 - the
attention logits are O(10), so fp8's ~4% relative error becomes a +-0.5
logit perturbation through exp and blows the 2e-2 gate (measured
4.3e-2). Activations quantize on the fly (LN2 rstd carries x16, the
FFN1 ReLU carries the descale+x32, the softmax-denominator ones-columns
carry 1/32 so O lands fp8 for free). DR stationaries are trimmed to 100
tokens: LDWEIGHTS cost is its column count.

Scheduling is built around the PE clock throttle (HAM): the PE runs at
1.2GHz until it sees ~3.4us of sustained streaming and re-throttles on
idle/low-activity windows. The attention phase (64-row scores, 100-row
AV, transposes) can never look busy enough, so the pipeline strictly
separates per group g:
  PA: attention(g) + ln1(g+2) chain   <- throttled but LDW/latency-bound
  PB: tail(g-1) + qkv(g+1), all dense <- one long 2.4GHz burst
A 32-matmul identity warm-up holds the activity window open through the
DMA/LN prologue. Small groups (4, 6 ... 4) sit at both ends where the
pipeline has no dense work to pair with attention. LN finish/apply run
in half-group chunks so the first transposes never wait on a full
group's stats; x DMAs are issued two phases ahead.

Engine placement: LN applies on GpSimd, LN stats/finish + attention
epilogue on DVE (both attention head-triples share one PSUM bank, so
softmax normalization is ONE reciprocal + ONE fused mul), Q/K/V/OT
PSUM->SBUF copies + exp/relu on the scalar engine (one ACT table set,
zero table swaps). The q/k/v/ffn biases are exactly zero for this
problem instance; the bias adds become plain copies.
"""

import numpy as np
from contextlib import ExitStack

B, T, D = 512, 100, 384
H, HS = 6, 64
DH = 4 * D                      # FFN hidden 1536
N_CORES = 8
BC = B // N_CORES               # batches per core
EPS = 1e-5
MASK_VAL = -1e9
NB = 10                         # batches per inner group
NTOK_MAX = 1024                 # QT/KT contiguous token width
KC_D = D // 128                 # 3 contraction chunks over D
KC_H = DH // 128                # 12 contraction chunks over DH
NJ_H = KC_H // 2                # 6 DoubleRow k-pairs over DH
VW = 66                         # V columns per head: 64 + 2 ones (denominator)

S_W = 64.0                      # fp8 weight scale (w1, w2, wproj)
S_X2 = 16.0                     # fp8 LN2-activation scale
S_H = 32.0                      # fp8 FFN-hidden scale
S_O = 32.0                      # fp8 attention-output scale

_NC_CACHE = {}


def _build_nc(use_bv, use_bp, use_b2, use_bq, use_b1):
    import concourse.bass as bass
    import concourse.tile as tile
    from concourse import bacc, mybir

    f32 = mybir.dt.float32
    bf16 = mybir.dt.bfloat16
    f8 = mybir.dt.float8e4
    AF = mybir.ActivationFunctionType
    OP = mybir.AluOpType
    PM = mybir.MatmulPerfMode
    ts = bass.ts

    nc = bacc.Bacc("TRN2", target_bir_lowering=False, debug=False,
                   enable_asserts=True, num_devices=N_CORES)

    x_d = nc.dram_tensor("x", [BC, T, D], f32, kind="ExternalInput").ap()
    wq_d = nc.dram_tensor("wq_l", [128, KC_D, D], bf16, kind="ExternalInput").ap()
    wk_d = nc.dram_tensor("wk_l", [128, KC_D, D], bf16, kind="ExternalInput").ap()
    wv_d = nc.dram_tensor("wv_l", [128, KC_D, D], bf16, kind="ExternalInput").ap()
    wp2_d = nc.dram_tensor("wp_dr", [128, 2, D], f8, kind="ExternalInput").ap()
    wpk_d = nc.dram_tensor("wp_k2", [128, D], f8, kind="ExternalInput").ap()
    w12_d = nc.dram_tensor("w1_dr", [128, KC_H, 2, 128], f8,
                           kind="ExternalInput").ap()
    w1k_d = nc.dram_tensor("w1_k2", [128, DH], f8, kind="ExternalInput").ap()
    w22_d = nc.dram_tensor("w2_dr", [128, NJ_H, 2, D], f8,
                           kind="ExternalInput").ap()
    mask_d = nc.dram_tensor("mask3", [T, 3 * T], bf16, kind="ExternalInput").ap()
    id_d = nc.dram_tensor("ident", [128, 128], bf16, kind="ExternalInput").ap()
    bq_d = bk_d = b1_d = bv_d = bp_d = b2_d = None
    if use_bq:
        bq_d = nc.dram_tensor("bq_l", [128, KC_D], f32, kind="ExternalInput").ap()
        bk_d = nc.dram_tensor("bk_l", [128, KC_D], f32, kind="ExternalInput").ap()
    if use_b1:
        b1_d = nc.dram_tensor("b1_l", [128, KC_H], f32, kind="ExternalInput").ap()
    if use_bv:
        bv_d = nc.dram_tensor("bv_bc", [128, D], f32, kind="ExternalInput").ap()
    if use_bp:
        bp_d = nc.dram_tensor("bp_bc", [128, D], f32, kind="ExternalInput").ap()
    if use_b2:
        b2_d = nc.dram_tensor("b2_bc", [128, D], f32, kind="ExternalInput").ap()
    out_d = nc.dram_tensor("out", [BC, T, D], f32, kind="ExternalOutput").ap()

    with tile.TileContext(nc) as tc, ExitStack() as ctx:
        wpool = ctx.enter_context(tc.tile_pool(name="wpool", bufs=1))
        px = ctx.enter_context(tc.tile_pool(name="px", bufs=38))
        pxn = ctx.enter_context(tc.tile_pool(name="pxn", bufs=4))
        pst = ctx.enter_context(tc.tile_pool(name="pst", bufs=4))
        pxt = ctx.enter_context(tc.tile_pool(name="pxt", bufs=2))
        pqk = ctx.enter_context(tc.tile_pool(name="pqk", bufs=2))
        pv = ctx.enter_context(tc.tile_pool(name="pv", bufs=2))
        patt = ctx.enter_context(tc.tile_pool(name="patt", bufs=4))
        po = ctx.enter_context(tc.tile_pool(name="po", bufs=20))
        pot = ctx.enter_context(tc.tile_pool(name="pot", bufs=2))
        px2 = ctx.enter_context(tc.tile_pool(name="px2", bufs=1))
        phid = ctx.enter_context(tc.tile_pool(name="phid", bufs=1))
        pout = ctx.enter_context(tc.tile_pool(name="pout", bufs=3))
        # PSUM: 8 banks total -> (pool,tag) bufs sum to 8
        psq = ctx.enter_context(tc.tile_pool(name="psq", bufs=2, space="PSUM"))
        pstk = ctx.enter_context(tc.tile_pool(name="pstk", bufs=2, space="PSUM"))
        psa = ctx.enter_context(tc.tile_pool(name="psa", bufs=4, space="PSUM"))

        # identity first: it gates the PE warm-up matmuls below
        id_sb = wpool.tile([128, 128], bf16)
        nc.sync.dma_start(out=id_sb[:], in_=id_d[:])
        # PE warm-up: HAM halves the PE clock until it sees ~3.4us of
        # sustained matmul activity, and any >3.4us idle re-throttles.
        # Burn the DMA/LN prologue keeping the activity window open so
        # the first real matmuls run at 2.4GHz.
        for _ in range(32):
            wt = psq.tile([128, 512], f32, tag="ps_big")
            nc.tensor.matmul(wt[:, 0:128], id_sb[:, :], id_sb[:, :],
                             start=True, stop=True, skip_group_check=True)

        # resident weights / constants, emitted via a generator so the
        # first group's x DMAs interleave ahead of the fp8 weights
        wq_sb = wpool.tile([128, KC_D, D], bf16)
        wk_sb = wpool.tile([128, KC_D, D], bf16)
        wv_sb = wpool.tile([128, KC_D, D], bf16)
        wp2_sb = wpool.tile([128, 2, D], f8)
        wpk_sb = wpool.tile([128, D], f8)
        w12_sb = wpool.tile([128, KC_H, 2, 128], f8)
        w1k_sb = wpool.tile([128, DH], f8)
        w22_sb = wpool.tile([128, NJ_H, 2, D], f8)
        mask_sb = wpool.tile([T, 3 * T], bf16)
        bq_sb = bk_sb = b1_sb = bv_sb = bp_sb = b2_sb = None
        if use_bq:
            bq_sb = wpool.tile([128, KC_D], f32)
            bk_sb = wpool.tile([128, KC_D], f32)
        if use_b1:
            b1_sb = wpool.tile([128, KC_H], f32)
        if use_bv:
            bv_sb = wpool.tile([128, D], f32)
        if use_bp:
            bp_sb = wpool.tile([128, D], f32)
        if use_b2:
            b2_sb = wpool.tile([128, D], f32)

        def emit_weights():
            # qkv weights first (gate the first matmuls), fp8/FFN last
            loads = [(wq_sb, wq_d), (wk_sb, wk_d), (wv_sb, wv_d),
                     (mask_sb, mask_d)]
            if use_bq:
                loads += [(bq_sb, bq_d), (bk_sb, bk_d)]
            if use_bv:
                loads += [(bv_sb, bv_d)]
            loads += [(wp2_sb, wp2_d), (wpk_sb, wpk_d)]
            if use_bp:
                loads += [(bp_sb, bp_d)]
            loads += [(w12_sb, w12_d), (w1k_sb, w1k_d), (w22_sb, w22_d)]
            if use_b1:
                loads += [(b1_sb, b1_d)]
            if use_b2:
                loads += [(b2_sb, b2_d)]
            for sb, dr in loads:
                nc.sync.dma_start(out=sb[:], in_=dr[:])
                yield

        assert D <= nc.vector.BN_STATS_FMAX
        # rsqrt seed polynomial (fit of v**-0.5 over [0.55, 1.65]; LN
        # variances are ~1 +/- 0.3, and 2 Newton steps make the result
        # robust to 5e-7 even well outside the fit range). eps=1e-5 is
        # negligible against var >= 0.5 and is dropped.
        RC2, RC1, RC0 = 0.35143019, -1.2540463, 1.90588191
        DS_QK = 1.0 / (S_X2 * S_W)      # qkv PSUM descale (xn x16, w x64)

        def ln_stats(x_sl, mvall, i):
            st = pst.tile([128, nc.vector.BN_STATS_DIM], f32, tag="bnst")
            nc.vector.bn_stats(out=st[:T], in_=x_sl)
            nc.vector.bn_aggr(out=mvall[:T, :, i], in_=st[:T])

        def ln_finish(mvall, w, tag, S, o=0):
            # rstd*S entirely on the DVE (poly seed + 2 Newton steps): the
            # scalar engine then only ever runs Exp/Relu/Copy, which share
            # one ACT table set -> zero table swaps.
            var = mvall[:T, 1, o:o + w]
            y = pst.tile([128, NB], f32, tag=tag + "y")
            u = pst.tile([128, NB], f32, tag=tag + "u")
            nc.vector.tensor_scalar(out=u[:T, :w], in0=var, scalar1=RC2,
                                    scalar2=RC1, op0=OP.mult, op1=OP.add)
            nc.vector.tensor_mul(u[:T, :w], u[:T, :w], var)
            nc.vector.tensor_scalar_add(out=y[:T, :w], in0=u[:T, :w],
                                        scalar1=RC0)
            nc.vector.tensor_mul(u[:T, :w], y[:T, :w], y[:T, :w])
            nc.vector.tensor_mul(u[:T, :w], u[:T, :w], var)
            nc.vector.tensor_scalar(out=u[:T, :w], in0=u[:T, :w],
                                    scalar1=-0.5 * S, scalar2=1.5 * S,
                                    op0=OP.mult, op1=OP.add)
            nc.vector.tensor_mul(y[:T, :w], y[:T, :w], u[:T, :w])
            nm = pst.tile([128, NB], f32, tag=tag + "nm")
            nc.vector.scalar_tensor_tensor(out=nm[:T, :w],
                                           in0=mvall[:T, 0, o:o + w],
                                           scalar=-1.0, in1=y[:T, :w],
                                           op0=OP.mult, op1=OP.mult)
            return y, nm

        def ln_apply(x_sl, rs2, nm2, j, tag):
            xn = pxn.tile([128, D], bf16, tag=tag)
            nc.gpsimd.tensor_scalar(out=xn[:T], in0=x_sl,
                                    scalar1=rs2[:T, j:j + 1],
                                    scalar2=nm2[:T, j:j + 1],
                                    op0=OP.mult, op1=OP.add)
            ps = psq.tile([128, KC_D, T], bf16, tag="ps_big")
            for c in range(KC_D):
                nc.tensor.transpose(ps[:128, c, :T],
                                    xn[:T, ts(c, 128)], id_sb[:T, :T])
            return ps

        xf = x_d.flatten_outer_dims()
        of = out_d.flatten_outer_dims()

        # small groups at both ends: the ramp-in attention phases run
        # before any FFN work exists to pair with (PE exposed at the
        # throttled clock), and the final tail drains alone - keep those
        # exposed phases short, full-size groups in the steady state
        groups = []
        tok = 0
        for nb in [4, 6] + [NB] * 5 + [4]:
            groups.append((tok, nb))
            tok += nb * T
        assert sum(nb for _, nb in groups) == BC

        live = {}

        def emit_ln1(gi):
            tok0, nb = groups[gi]
            XnT = pxt.tile([128, KC_D, NTOK_MAX], bf16, tag="xnt")
            mvall = pst.tile([128, nc.vector.BN_AGGR_DIM, NB], f32, tag="xn1mv")
            xres = []
            for i in range(nb):
                xt_ = px.tile([128, D], f32, tag="xres")
                nc.sync.dma_start(out=xt_[:T],
                                  in_=xf[tok0 + i * T: tok0 + (i + 1) * T])
                xres.append(xt_)
            # finish/apply in small chunks (4+3+3 for full groups): each
            # apply burst only waits on its own chunk's DMAs+stats, and
            # the serial gpsimd/transpose bursts stay short so they never
            # dam the in-order PE queue for long
            cuts = [0, 4, 7, nb] if nb == NB else [0, (nb + 1) // 2, nb]
            ln_stats(xres[0][:T], mvall, 0)
            yield
            for ci in range(len(cuts) - 1):
                lo, hi = cuts[ci], cuts[ci + 1]
                for i in range(max(1, lo), hi):
                    ln_stats(xres[i][:T], mvall, i)
                y_nm = ln_finish(mvall, hi - lo, f"xn1{ci}", 1.0, o=lo)
                yield
                for i in range(lo, hi):
                    ps = ln_apply(xres[i][:T], *y_nm, i - lo, "xn1")
                    nc.scalar.activation(out=XnT[:, :, i * T:(i + 1) * T],
                                         in_=ps[:, :, :T], func=AF.Copy)
                    yield
            live[gi] = dict(xres=xres, XnT=XnT)
            yield

        def emit_qkv(gi):
            """Q/K/V projections for group gi. Generator (PE-dense filler)."""
            tok0, nb = groups[gi]
            ntok = nb * T
            XnT = live[gi]["XnT"]
            QT = pqk.tile([128, KC_D, NTOK_MAX], bf16, tag="qt")
            KT = pqk.tile([128, KC_D, NTOK_MAX], bf16, tag="kt")
            for di, (dst, w_sb, b_sb) in enumerate(((QT, wq_sb, bq_sb),
                                                    (KT, wk_sb, bk_sb))):
                slabs = [(h0, min(h0 + 500, ntok))
                         for h0 in range(0, ntok, 500)]
                for m in range(KC_D):
                    # both slabs' chains interleave across the two psum
                    # banks: each scalar copy gets a full chain of runway
                    grp = slabs[:2]
                    pss_ = [psq.tile([128, 512], f32, tag="ps_big",
                                     name=f"ps_qk{k}")
                            for k in range(len(grp))]
                    for kc in range(KC_D):
                        for k, (h0, h1) in enumerate(grp):
                            nc.tensor.matmul(pss_[k][:, :h1 - h0],
                                             w_sb[:, kc, ts(m, 128)],
                                             XnT[:, kc, h0:h1],
                                             start=(kc == 0),
                                             stop=(kc == KC_D - 1))
                    for k, (h0, h1) in enumerate(grp):
                        if b_sb is not None:
                            nc.vector.tensor_scalar_add(
                                out=dst[:, m, h0:h1], in0=pss_[k][:, :h1 - h0],
                                scalar1=b_sb[:, m:m + 1])
                        else:
                            nc.scalar.activation(out=dst[:, m, h0:h1],
                                                 in_=pss_[k][:, :h1 - h0],
                                                 func=AF.Copy)
                    yield
            V = pv.tile([128, NB, H, VW], bf16, tag="v")
            # ones columns carry 1/S_O so the softmax denominator comes out
            # pre-scaled: rden = S_O/den, making O fp8 at scale S_O free.
            nc.gpsimd.memset(V[:T, :nb, :, 64:VW], 1.0 / S_O)
            for b0 in range(0, nb, 2):
                pr = [b0] + ([b0 + 1] if b0 + 1 < nb else [])
                pss_ = [pstk.tile([128, D], f32, tag="ps_tok",
                                  name=f"ps_v{b}") for b in pr]
                for kc in range(KC_D):
                    for k, b in enumerate(pr):
                        nc.tensor.matmul(pss_[k][:T, :],
                                         XnT[:, kc, b * T:(b + 1) * T],
                                         wv_sb[:, kc, :],
                                         start=(kc == 0),
                                         stop=(kc == KC_D - 1))
                for k, b in enumerate(pr):
                    psh = pss_[k][:T].rearrange("p (h s) -> p h s", h=H)
                    if use_bv:
                        bvh = bv_sb[:T].rearrange("p (h s) -> p h s", h=H)
                        nc.vector.tensor_add(V[:T, b, :, 0:64], psh, bvh)
                    else:
                        nc.scalar.activation(out=V[:T, b, :, 0:64], in_=psh,
                                             func=AF.Copy)
                yield
            live[gi].update(QT=QT, KT=KT, V=V)
            yield

        def emit_attn(gi):
            """Attention for group gi -> per-batch O (bf16, [tok, chan])."""
            tok0, nb = groups[gi]
            QT, KT, V = live[gi]["QT"], live[gi]["KT"], live[gi]["V"]

            def front(b):
                bs = slice(b * T, (b + 1) * T)
                # emit head pairs (even, odd) interleaved: the two PE
                # row-groups (rows 0:64 / 64:128, via auto tile_position)
                # stream concurrently. Same-bank writers (h, h+2, h+4)
                # stay same-row-group, so they serialize - no concurrent
                # same-bank PSUM writes.
                # mask-add on the DVE: in the strict phasing the PE is the
                # critical engine and the attention phase runs throttled,
                # so +2 DVE ops/batch beat +2 PE preload matmuls/batch
                exs = []
                for g3 in range(2):
                    ps_s = psa.tile([128, 3, T], f32, tag="ps_att")
                    for j, h in enumerate((g3, g3 + 2, g3 + 4)):
                        mb, mo = h // 2, (h % 2) * 64
                        nc.tensor.matmul(ps_s[:T, j, :],
                                         KT[mo:mo + 64, mb, bs],
                                         QT[mo:mo + 64, mb, bs],
                                         start=True, stop=True)
                    sm = patt.tile([128, 3 * T], f32, tag="sm")
                    nc.vector.tensor_add(sm[:T], ps_s[:T, :, :], mask_sb[:T])
                    ex = patt.tile([128, 3 * T], bf16, tag="ex")
                    nc.scalar.activation(out=ex[:T], in_=sm[:T], func=AF.Exp)
                    exs.append(ex)
                return exs

            def back(b, exs):
                # both head-triples share one PSUM bank (2x3x66 fits), so
                # the normalization collapses to ONE reciprocal + ONE mul
                O_sb = po.tile([128, D], bf16, tag="o")
                rden = pst.tile([128, H], f32, tag="rden")
                ps_o = psa.tile([128, 2, 3, VW], f32, tag="ps_att")
                for g3 in range(2):
                    ex = exs[g3]
                    for j in range(3):
                        nc.tensor.matmul(ps_o[:T, g3, j, :],
                                         ex[:T, ts(j, T)],
                                         V[:T, b, g3 + 2 * j, :],
                                         start=True, stop=True)
                nc.vector.reciprocal(out=rden[:T, :], in_=ps_o[:T, :, :, 64:65])
                rsl = rden[:T].rearrange("p (g j) -> p j g", g=2)
                rb = bass.AP(tensor=rsl.tensor, offset=rsl.offset,
                             ap=[*map(list, rsl.ap), [0, 64]])
                osl = O_sb[:T].rearrange("p (c two s) -> p c two s",
                                         two=2, s=64)
                nc.vector.tensor_mul(
                    osl, ps_o[:T].rearrange("p g c v -> p c g v")[:, :, :, 0:64],
                    rb)
                return O_sb

            prev = None
            Os = []
            for b in range(nb):
                exs = front(b)
                if prev is not None:
                    Os.append(back(prev, prev_exs))
                prev, prev_exs = b, exs
                yield
            Os.append(back(prev, prev_exs))
            live[gi]["O"] = Os

        def emit_tail(gi):
            """O transpose, proj+residual, LN2, FFN, store for group gi."""
            tok0, nb = groups[gi]
            ntok = nb * T
            xres, Os = live[gi]["xres"], live[gi]["O"]
            OT = pot.tile([128, NB, KC_D, 128], f8, tag="ot")
            X2 = px2.tile([128, NB, D], f32, tag="x2")
            mvall = pst.tile([128, nc.vector.BN_AGGR_DIM, NB], f32, tag="xn2mv")
            Xn2T = pxt.tile([128, KC_D, NTOK_MAX], f8, tag="xn2t")
            DS_P = 1.0 / (S_O * S_W)    # proj PSUM descale
            DS_1 = S_H / (S_X2 * S_W)   # FFN1 PSUM descale * hidden scale
            DS_2 = 1.0 / (S_H * S_W)    # FFN2 PSUM descale

            def o_transpose(i):
                # deferred from the attention phase: runs in the dense
                # phase where the PE clock is warm and it has runway
                ps_t = psa.tile([128, KC_D, T], bf16, tag="ps_att")
                for c in range(KC_D):
                    nc.tensor.transpose(ps_t[:128, c, :],
                                        Os[i][:T, ts(c, 128)], id_sb[:T, :T])
                nc.scalar.activation(out=OT[:, i, :, 0:T], in_=ps_t[:, :, :],
                                     func=AF.Copy)

            def fin_app(i):
                # chunk A = batches 0-4 == FFN1's first token slab, so
                # slab-0 FFN1 matmuls can be pulled into the proj loop
                if i == min(4, nb - 1):
                    w = i + 1
                    y_nm = ln_finish(mvall, w, "xn2a", S_X2)
                    for k in range(w):
                        ps = ln_apply(X2[:T, k, :], y_nm[0], y_nm[1], k, "xn2")
                        nc.vector.tensor_copy(Xn2T[:, :, k * T:(k + 1) * T],
                                              ps[:, :, :T])
                elif nb == 10 and i == 7:
                    y_nm = ln_finish(mvall, 3, "xn2c", S_X2, o=5)
                    for k in range(5, 8):
                        ps = ln_apply(X2[:T, k, :], y_nm[0], y_nm[1], k - 5,
                                      "xn2")
                        nc.vector.tensor_copy(Xn2T[:, :, k * T:(k + 1) * T],
                                              ps[:, :, :T])
                elif i == nb - 1 and nb > 5:
                    o = 8 if nb == 10 else 5
                    y_nm = ln_finish(mvall, nb - o, "xn2b", S_X2, o=o)
                    for k in range(o, nb):
                        ps = ln_apply(X2[:T, k, :], y_nm[0], y_nm[1], k - o,
                                      "xn2")
                        nc.vector.tensor_copy(Xn2T[:, :, k * T:(k + 1) * T],
                                              ps[:, :, :T])

            def ffn1_slab(h0, h1, m):
                ps = psq.tile([128, 512], f32, tag="ps_big")
                nc.tensor.matmul(ps[:, :h1 - h0], w12_sb[:, m, :, :],
                                 Xn2T[:, 0:2, h0:h1],
                                 start=True, stop=False,
                                 perf_mode=PM.DoubleRow)
                nc.tensor.matmul(ps[:, :h1 - h0], w1k_sb[:, ts(m, 128)],
                                 Xn2T[:, 2, h0:h1],
                                 start=False, stop=True)
                b0 = h0 // T
                nbh = (h1 - h0) // T
                dsth = HT[:, m // 2, b0:b0 + nbh, m % 2, 0:T]
                psb = ps[:, 0:nbh * T].rearrange("p (b t) -> p b t", t=T)
                if use_b1:
                    nc.scalar.activation(out=dsth, in_=psb, func=AF.Relu,
                                         bias=b1_sb[:, m:m + 1], scale=DS_1)
                else:
                    nc.scalar.activation(out=dsth, in_=psb, func=AF.Relu,
                                         scale=DS_1)

            HT = phid.tile([128, NJ_H, NB, 2, 128], f8, tag="hid")
            o_transpose(0)
            if nb > 1:
                o_transpose(1)
            # proj pair-pipelined like FFN2: both psum banks' short
            # DR chains interleave so the DVE epilogue has runway
            for i0 in range(0, nb, 2):
                pr = [i0] + ([i0 + 1] if i0 + 1 < nb else [])
                for i in (i0 + 2, i0 + 3):
                    if i < nb:
                        o_transpose(i)
                pss_ = [pstk.tile([128, D], f32, tag="ps_tok",
                                  name=f"ps_pj_{i}") for i in pr]
                # stationary trimmed to T tokens: LDWEIGHTS cost is its
                # column count, so the 28 pad columns are pure overhead
                for k, i in enumerate(pr):
                    nc.tensor.matmul(pss_[k][0:T, :], OT[:, i, 0:2, 0:T],
                                     wp2_sb[:, :, :], start=True, stop=False,
                                     perf_mode=PM.DoubleRow)
                for k, i in enumerate(pr):
                    nc.tensor.matmul(pss_[k][0:T, :], OT[:, i, 2, 0:T],
                                     wpk_sb[:, :], start=False, stop=True)
                for k, i in enumerate(pr):
                    ps = pss_[k]
                    if use_bp:
                        nc.vector.scalar_tensor_tensor(out=ps[:T, :],
                                                       in0=ps[:T, :],
                                                       scalar=DS_P,
                                                       in1=bp_sb[:T, :],
                                                       op0=OP.mult, op1=OP.add)
                        nc.vector.tensor_add(X2[:T, i, :], ps[:T, :],
                                             xres[i][:T, :])
                    else:
                        nc.vector.scalar_tensor_tensor(out=X2[:T, i, :],
                                                       in0=ps[:T, :],
                                                       scalar=DS_P,
                                                       in1=xres[i][:T, :],
                                                       op0=OP.mult, op1=OP.add)
                    ln_stats(X2[:T, i, :], mvall, i)
                    fin_app(i)
                    # slab-0 FFN1 pulled forward: ready once chunk A is
                    # applied, it fills the PE while chunk B normalizes
                    if nb == 10 and i >= 6:
                        for m in range(3 * (i - 6), 3 * (i - 5)):
                            ffn1_slab(0, 500, m)
                    yield
            # FFN2 software-pipelined across batch pairs: the two psum
            # banks' accumulation chains interleave, so the DVE epilogue
            # of batch i has a full 12-matmul window before its bank is
            # reallocated (with per-batch chains the WAR hit every 6)
            def ffn2_pair(i0):
                pr = [i0] + ([i0 + 1] if i0 + 1 < nb else [])
                pss_ = [pstk.tile([128, D], f32, tag="ps_tok",
                                  name=f"ps_f2_{i}") for i in pr]
                for j in range(NJ_H):
                    for k in range(len(pr)):
                        nc.tensor.matmul(pss_[k][0:T, :],
                                         HT[:, j, pr[k], :, 0:T],
                                         w22_sb[:, j, :, :],
                                         start=(j == 0),
                                         stop=(j == NJ_H - 1),
                                         perf_mode=PM.DoubleRow)
                for k, i in enumerate(pr):
                    ps = pss_[k]
                    ot_ = pout.tile([128, D], f32, tag="outt")
                    if use_b2:
                        nc.vector.scalar_tensor_tensor(out=ps[:T, :],
                                                       in0=ps[:T, :],
                                                       scalar=DS_2,
                                                       in1=b2_sb[:T, :],
                                                       op0=OP.mult, op1=OP.add)
                        nc.vector.tensor_add(ot_[:T, :], ps[:T, :],
                                             X2[:T, i, :])
                    else:
                        nc.vector.scalar_tensor_tensor(out=ot_[:T, :],
                                                       in0=ps[:T, :],
                                                       scalar=DS_2,
                                                       in1=X2[:T, i, :],
                                                       op0=OP.mult, op1=OP.add)
                    nc.sync.dma_start(out=of[tok0 + i * T: tok0 + (i + 1) * T],
                                      in_=ot_[:T, :])

            # slab-0 batches' FFN2 only needs slab-0 HT (complete after
            # the proj loop) - interleave those pairs into the slab-1
            # FFN1 emission so the FFN boundary never drains the PE
            done2 = 0
            for m in range(KC_H):
                for h0 in range(500 if nb == 10 else 0, ntok, 500):
                    ffn1_slab(h0, min(h0 + 500, ntok), m)
                if nb == 10 and m in (3, 7):
                    ffn2_pair(2 * done2)
                    done2 += 1
                if m % 2 == 1:
                    yield
            for i0 in range(2 * done2, nb, 2):
                ffn2_pair(i0)
                yield
            del live[gi]

        def alternate(*its, weights=None):
            its = [it for it in its if it is not None]
            if weights is None:
                weights = [1] * len(its)
            w = {id(it): wt for it, wt in zip(its, weights)}
            while its:
                for it in list(its):
                    for _ in range(w.get(id(it), 1)):
                        try:
                            next(it)
                        except StopIteration:
                            its.remove(it)
                            break

        def drive(it, steps):
            # advance a generator a bounded number of yields; True if done
            for _ in range(steps):
                try:
                    next(it)
                except StopIteration:
                    return True
            return False

        # software pipeline, per iteration g:
        #   P1: [QKV(g) x LN1(g+1)]   - dense Q/K streams hide LN1(g+1)'s
        #       serial finish/apply chain, so qkv(g+1) never stalls on XnT
        #   P2: [tail(g-1) x attn(g)] - FFN streams hide attention's gaps,
        #       attention matmuls hide the LN2 chain inside tail
        # Strict phase separation, driven by PE clock throttling (HAM):
        # the attention phase's small matmuls can't keep the activity
        # window busy, so it always runs at the 1.2GHz throttled clock -
        # but it is LDWEIGHTS/latency-bound and barely cares. The dense
        # full-row streaming work (qkv + proj/FFN) DOES care, so it is
        # kept out of the attention windows entirely:
        #   PA: attn(g) [+ ln1(g+2) chain: DVE/gpsimd work, PE only sees
        #       its clock-insensitive transposes]
        #   PB: tail(g-1) + qkv(g+1), all dense, one long K=8/8 burst
        ng = len(groups)
        alternate(emit_ln1(0), emit_weights())
        alternate(emit_qkv(0), emit_ln1(1), weights=[2, 1])
        for g in range(ng - 1):
            alternate(emit_attn(g),
                      emit_ln1(g + 2) if g + 2 < ng else None)
            alternate(emit_tail(g - 1) if g >= 1 else None,
                      emit_qkv(g + 1))
        # drain: the last attention rides with the second-to-last tail
        # (nothing dense is left to protect from HAM), and the last tail
        # interleaves with it batch-by-batch
        alternate(emit_attn(ng - 1), emit_tail(ng - 2))
        alternate(emit_tail(ng - 1))

    nc.compile()
    return nc


def _get_nc(*flags):
    if flags not in _NC_CACHE:
        _NC_CACHE[flags] = _build_nc(*flags)
    return _NC_CACHE[flags]


def _prep_inputs(x, wq, wk, wv, wproj, bproj, w1, b1, w2, b2, g1, beta1, g2, beta2):
    import ml_dtypes
    f = np.float32
    bf = ml_dtypes.bfloat16
    f8 = ml_dtypes.float8_e4m3
    # stack per-head projections into [D, D] with head h at columns h*HS:(h+1)*HS
    wq_f = np.ascontiguousarray(wq.transpose(1, 0, 2).reshape(D, D), dtype=f)
    wk_f = np.ascontiguousarray(wk.transpose(1, 0, 2).reshape(D, D), dtype=f)
    wv_f = np.ascontiguousarray(wv.transpose(1, 0, 2).reshape(D, D), dtype=f)
    scale = np.float32(HS ** 0.5)
    # fold LN1 affine into qkv weights, LN2 affine into w1
    wq_p = (g1[:, None] * wq_f) * scale
    wk_p = g1[:, None] * wk_f
    wv_p = g1[:, None] * wv_f
    w1_p = g2[:, None] * w1
    bq = (beta1 @ wq_f) * scale
    bk = beta1 @ wk_f
    bv = beta1 @ wv_f
    b1_p = b1 + beta2 @ w1
    bp = bproj
    b2_p = b2

    def lay(w, kc, dt, s=1.0):
        # [K, M] -> [128, kc, M] with K split into kc chunks of 128
        wl = np.asarray(w, dtype=f) * s
        return np.ascontiguousarray(
            wl.reshape(kc, 128, w.shape[1]).transpose(1, 0, 2)
        ).astype(dt)

    def q8(w):
        return np.clip(np.asarray(w, f) * S_W, -448.0, 448.0).astype(f8)

    def layb(bias, kc, s=1.0):
        return np.ascontiguousarray((bias * s).reshape(kc, 128).T, dtype=f)

    # transposed causal mask, tiled for 3 heads: keep (t >= u)
    maskT = np.full((T, T), MASK_VAL, dtype=f)
    maskT[np.triu_indices(T)] = 0.0
    mask3 = np.ascontiguousarray(np.tile(maskT, (1, 3))).astype(bf)

    w2_q = q8(w2)                     # [1536, 384]
    wp_q = q8(wproj)                  # [384, 384]
    w1_q = q8(w1_p)                   # [384, 1536]
    # DoubleRow planar-pair layouts (k-tile pair per partition)
    wp_dr = np.ascontiguousarray(
        wp_q.reshape(KC_D, 128, D)[0:2].transpose(1, 0, 2))      # [128,2,384]
    wp_k2 = np.ascontiguousarray(wp_q[256:384, :])               # [128,384]
    w1_dr = np.ascontiguousarray(
        w1_q.reshape(KC_D, 128, KC_H, 128)[0:2]
        .transpose(1, 2, 0, 3))                                  # [128,12,2,128]
    w1_k2 = np.ascontiguousarray(w1_q[256:384, :])               # [128,1536]
    w2_dr = np.ascontiguousarray(
        w2_q.reshape(NJ_H, 2, 128, D).transpose(2, 0, 1, 3))     # [128,6,2,384]

    shared = {
        "wq_l": lay(wq_p, KC_D, bf), "wk_l": lay(wk_p, KC_D, bf),
        "wv_l": lay(wv_p, KC_D, bf),
        "wp_dr": wp_dr, "wp_k2": wp_k2, "w1_dr": w1_dr, "w1_k2": w1_k2,
        "w2_dr": w2_dr,
        "mask3": mask3, "ident": np.eye(128, dtype=f).astype(bf),
    }
    use_bv = bool(np.any(bv))
    use_bp = bool(np.any(bp))
    use_b2 = bool(np.any(b2_p))
    use_bq = bool(np.any(bq)) or bool(np.any(bk))
    use_b1 = bool(np.any(b1_p))
    if use_bq:
        shared["bq_l"] = layb(bq, KC_D)
        shared["bk_l"] = layb(bk, KC_D)
    if use_b1:
        # the ACT applies relu(ps*DS_1 + bias): bias must carry S_H
        shared["b1_l"] = layb(b1_p, KC_H, S_H)
    if use_bv:
        shared["bv_bc"] = np.ascontiguousarray(np.tile(bv.astype(f), (128, 1)))
    if use_bp:
        shared["bp_bc"] = np.ascontiguousarray(np.tile(np.asarray(bp, f), (128, 1)))
    if use_b2:
        shared["b2_bc"] = np.ascontiguousarray(np.tile(np.asarray(b2_p, f), (128, 1)))
    return shared, (use_bv, use_bp, use_b2, use_bq, use_b1)


def kernel(**inputs):
    from concourse.bass_utils import run_bass_kernel_spmd

    x = np.asarray(inputs["x"], dtype=np.float32)
    shared, flags = _prep_inputs(
        x, *[np.asarray(inputs[k], dtype=np.float32) for k in
             ("wq", "wk", "wv", "wproj", "bproj", "w1", "b1", "w2", "b2",
              "g1", "beta1", "g2", "beta2")])
    nc = _get_nc(*flags)
    in_maps = []
    for c in range(N_CORES):
        m = dict(shared)
        m["x"] = np.ascontiguousarray(x[c * BC:(c + 1) * BC])
        in_maps.append(m)
    res = run_bass_kernel_spmd(nc, in_maps, core_ids=list(range(N_CORES)))
    return np.concatenate([res.results[i]["out"] for i in range(N_CORES)], axis=0)



# revision 83
# speedup vs baseline: 1.0208x; 1.0208x over previous
"""Trainium2 Bass kernel for a dense pre-norm transformer block (v3).

Reference semantics (per batch b, seq len T=100, d_model D=384, heads H=6):
  h   = LN(x) * g1 + beta1
  q,k,v = per-head projections of h;  wei = softmax(mask(q k^T * sqrt(64)))
  x2  = x + (wei v) Wp + bp
  out = x2 + relu(LN(x2)*g2+beta2 @ W1 + b1) @ W2 + b2

Distribution: data-parallel over the batch dim across 8 NeuronCores
(64 batches each); weights are replicated. No collectives.

Numerics: proj/FFN1/FFN2 run fp8(e4m3) with MatmulPerfMode.DoubleRow
(k-tile pairs, packed [128, ..., 2, X] layouts); Q/K/V stay bf16 - the
attention logits are O(10), so fp8's ~4% relative error becomes a +-0.5
logit perturbation through exp and blows the 2e-2 gate (measured
4.3e-2). Activations quantize on the fly (LN2 rstd carries x16, the
FFN1 ReLU carries the descale+x32, the softmax-denominator ones-columns
carry 1/32 so O lands fp8 for free). DR stationaries are trimmed to 100
tokens: LDWEIGHTS cost is its column count.

Scheduling is built around the PE clock throttle (HAM): the PE runs at
1.2GHz until it sees ~3.4us of sustained streaming and re-throttles on
idle/low-activity windows. The attention phase (64-row scores, 100-row
AV, transposes) can never look busy enough, so the pipeline strictly
separates per group g:
  PA: attention(g) + ln1(g+2) chain   <- throttled but LDW/latency-bound
  PB: tail(g-1) + qkv(g+1), all dense <- one long 2.4GHz burst
A 32-matmul identity warm-up holds the activity window open through the
DMA/LN prologue. Small groups (4, 6 ... 4) sit at both ends where the
pipeline has no dense work to pair with attention. LN finish/apply run
in half-group chunks so the first transposes never wait on a full
group's stats; x DMAs are issued two phases ahead.

Engine placement: LN applies on GpSimd, LN stats/finish + attention
epilogue on DVE (both attention head-triples share one PSUM bank, so
softmax normalization is ONE reciprocal + ONE fused mul), Q/K/V/OT
PSUM->SBUF copies + exp/relu on the scalar engine (one ACT table set,
zero table swaps). The q/k/v/ffn biases are exactly zero for this
problem instance; the bias adds become plain copies.
"""

import numpy as np
from contextlib import ExitStack

B, T, D = 512, 100, 384
H, HS = 6, 64
DH = 4 * D                      # FFN hidden 1536
N_CORES = 8
BC = B // N_CORES               # batches per core
EPS = 1e-5
MASK_VAL = -1e9
NB = 10                         # batches per inner group
NTOK_MAX = 1024                 # QT/KT contiguous token width
KC_D = D // 128                 # 3 contraction chunks over D
KC_H = DH // 128                # 12 contraction chunks over DH
NJ_H = KC_H // 2                # 6 DoubleRow k-pairs over DH
VW = 66                         # V columns per head: 64 + 2 ones (denominator)

S_W = 64.0                      # fp8 weight scale (w1, w2, wproj)
S_X2 = 16.0                     # fp8 LN2-activation scale
S_H = 32.0                      # fp8 FFN-hidden scale
S_O = 32.0                      # fp8 attention-output scale

_NC_CACHE = {}


def _build_nc(use_bv, use_bp, use_b2, use_bq, use_b1):
    import concourse.bass as bass
    import concourse.tile as tile
    from concourse import bacc, mybir

    f32 = mybir.dt.float32
    bf16 = mybir.dt.bfloat16
    f8 = mybir.dt.float8e4
    AF = mybir.ActivationFunctionType
    OP = mybir.AluOpType
    PM = mybir.MatmulPerfMode
    ts = bass.ts

    nc = bacc.Bacc("TRN2", target_bir_lowering=False, debug=False,
                   enable_asserts=True, num_devices=N_CORES)

    x_d = nc.dram_tensor("x", [BC, T, D], f32, kind="ExternalInput").ap()
    wq_d = nc.dram_tensor("wq_l", [128, KC_D, D], bf16, kind="ExternalInput").ap()
    wk_d = nc.dram_tensor("wk_l", [128, KC_D, D], bf16, kind="ExternalInput").ap()
    wv_d = nc.dram_tensor("wv_l", [128, KC_D, D], bf16, kind="ExternalInput").ap()
    wp2_d = nc.dram_tensor("wp_dr", [128, 2, D], f8, kind="ExternalInput").ap()
    wpk_d = nc.dram_tensor("wp_k2", [128, D], f8, kind="ExternalInput").ap()
    w12_d = nc.dram_tensor("w1_dr", [128, KC_H, 2, 128], f8,
                           kind="ExternalInput").ap()
    w1k_d = nc.dram_tensor("w1_k2", [128, DH], f8, kind="ExternalInput").ap()
    w22_d = nc.dram_tensor("w2_dr", [128, NJ_H, 2, D], f8,
                           kind="ExternalInput").ap()
    mask_d = nc.dram_tensor("mask3", [T, 3 * T], bf16, kind="ExternalInput").ap()
    id_d = nc.dram_tensor("ident", [128, 128], bf16, kind="ExternalInput").ap()
    bq_d = bk_d = b1_d = bv_d = bp_d = b2_d = None
    if use_bq:
        bq_d = nc.dram_tensor("bq_l", [128, KC_D], f32, kind="ExternalInput").ap()
        bk_d = nc.dram_tensor("bk_l", [128, KC_D], f32, kind="ExternalInput").ap()
    if use_b1:
        b1_d = nc.dram_tensor("b1_l", [128, KC_H], f32, kind="ExternalInput").ap()
    if use_bv:
        bv_d = nc.dram_tensor("bv_bc", [128, D], f32, kind="ExternalInput").ap()
    if use_bp:
        bp_d = nc.dram_tensor("bp_bc", [128, D], f32, kind="ExternalInput").ap()
    if use_b2:
        b2_d = nc.dram_tensor("b2_bc", [128, D], f32, kind="ExternalInput").ap()
    out_d = nc.dram_tensor("out", [BC, T, D], f32, kind="ExternalOutput").ap()

    with tile.TileContext(nc) as tc, ExitStack() as ctx:
        wpool = ctx.enter_context(tc.tile_pool(name="wpool", bufs=1))
        px = ctx.enter_context(tc.tile_pool(name="px", bufs=40))
        pxn = ctx.enter_context(tc.tile_pool(name="pxn", bufs=2))
        pst = ctx.enter_context(tc.tile_pool(name="pst", bufs=4))
        pxt = ctx.enter_context(tc.tile_pool(name="pxt", bufs=2))
        pqk = ctx.enter_context(tc.tile_pool(name="pqk", bufs=2))
        pv = ctx.enter_context(tc.tile_pool(name="pv", bufs=2))
        patt = ctx.enter_context(tc.tile_pool(name="patt", bufs=4))
        po = ctx.enter_context(tc.tile_pool(name="po", bufs=20))
        pot = ctx.enter_context(tc.tile_pool(name="pot", bufs=2))
        px2 = ctx.enter_context(tc.tile_pool(name="px2", bufs=1))
        phid = ctx.enter_context(tc.tile_pool(name="phid", bufs=1))
        pout = ctx.enter_context(tc.tile_pool(name="pout", bufs=3))
        # PSUM: 8 banks total -> (pool,tag) bufs sum to 8
        psq = ctx.enter_context(tc.tile_pool(name="psq", bufs=2, space="PSUM"))
        pstk = ctx.enter_context(tc.tile_pool(name="pstk", bufs=2, space="PSUM"))
        psa = ctx.enter_context(tc.tile_pool(name="psa", bufs=4, space="PSUM"))

        # identity first: it gates the PE warm-up matmuls below
        id_sb = wpool.tile([128, 128], bf16)
        nc.sync.dma_start(out=id_sb[:], in_=id_d[:])
        # PE warm-up: HAM halves the PE clock until it sees ~3.4us of
        # sustained matmul activity, and any >3.4us idle re-throttles.
        # Burn the DMA/LN prologue keeping the activity window open so
        # the first real matmuls run at 2.4GHz.
        for _ in range(32):
            wt = psq.tile([128, 512], f32, tag="ps_big")
            nc.tensor.matmul(wt[:, 0:128], id_sb[:, :], id_sb[:, :],
                             start=True, stop=True, skip_group_check=True)

        # resident weights / constants, emitted via a generator so the
        # first group's x DMAs interleave ahead of the fp8 weights
        wq_sb = wpool.tile([128, KC_D, D], bf16)
        wk_sb = wpool.tile([128, KC_D, D], bf16)
        wv_sb = wpool.tile([128, KC_D, D], bf16)
        wp2_sb = wpool.tile([128, 2, D], f8)
        wpk_sb = wpool.tile([128, D], f8)
        w12_sb = wpool.tile([128, KC_H, 2, 128], f8)
        w1k_sb = wpool.tile([128, DH], f8)
        w22_sb = wpool.tile([128, NJ_H, 2, D], f8)
        mask_sb = wpool.tile([T, 3 * T], bf16)
        bq_sb = bk_sb = b1_sb = bv_sb = bp_sb = b2_sb = None
        if use_bq:
            bq_sb = wpool.tile([128, KC_D], f32)
            bk_sb = wpool.tile([128, KC_D], f32)
        if use_b1:
            b1_sb = wpool.tile([128, KC_H], f32)
        if use_bv:
            bv_sb = wpool.tile([128, D], f32)
        if use_bp:
            bp_sb = wpool.tile([128, D], f32)
        if use_b2:
            b2_sb = wpool.tile([128, D], f32)

        def emit_weights():
            # qkv weights first (gate the first matmuls), fp8/FFN last
            loads = [(wq_sb, wq_d), (wk_sb, wk_d), (wv_sb, wv_d),
                     (mask_sb, mask_d)]
            if use_bq:
                loads += [(bq_sb, bq_d), (bk_sb, bk_d)]
            if use_bv:
                loads += [(bv_sb, bv_d)]
            loads += [(wp2_sb, wp2_d), (wpk_sb, wpk_d)]
            if use_bp:
                loads += [(bp_sb, bp_d)]
            loads += [(w12_sb, w12_d), (w1k_sb, w1k_d), (w22_sb, w22_d)]
            if use_b1:
                loads += [(b1_sb, b1_d)]
            if use_b2:
                loads += [(b2_sb, b2_d)]
            for sb, dr in loads:
                nc.sync.dma_start(out=sb[:], in_=dr[:])
                yield

        assert D <= nc.vector.BN_STATS_FMAX
        # rsqrt seed polynomial (fit of v**-0.5 over [0.55, 1.65]; LN
        # variances are ~1 +/- 0.3, and 2 Newton steps make the result
        # robust to 5e-7 even well outside the fit range). eps=1e-5 is
        # negligible against var >= 0.5 and is dropped.
        RC2, RC1, RC0 = 0.35143019, -1.2540463, 1.90588191
        DS_QK = 1.0 / (S_X2 * S_W)      # qkv PSUM descale (xn x16, w x64)

        def ln_stats(x_sl, mvall, i):
            st = pst.tile([128, nc.vector.BN_STATS_DIM], f32, tag="bnst")
            nc.vector.bn_stats(out=st[:T], in_=x_sl)
            nc.vector.bn_aggr(out=mvall[:T, :, i], in_=st[:T])

        def ln_finish(mvall, w, tag, S, o=0):
            # rstd*S entirely on the DVE (poly seed + 2 Newton steps): the
            # scalar engine then only ever runs Exp/Relu/Copy, which share
            # one ACT table set -> zero table swaps.
            var = mvall[:T, 1, o:o + w]
            y = pst.tile([128, NB], f32, tag=tag + "y")
            u = pst.tile([128, NB], f32, tag=tag + "u")
            nc.vector.tensor_scalar(out=u[:T, :w], in0=var, scalar1=RC2,
                                    scalar2=RC1, op0=OP.mult, op1=OP.add)
            nc.vector.tensor_mul(u[:T, :w], u[:T, :w], var)
            nc.vector.tensor_scalar_add(out=y[:T, :w], in0=u[:T, :w],
                                        scalar1=RC0)
            nc.vector.tensor_mul(u[:T, :w], y[:T, :w], y[:T, :w])
            nc.vector.tensor_mul(u[:T, :w], u[:T, :w], var)
            nc.vector.tensor_scalar(out=u[:T, :w], in0=u[:T, :w],
                                    scalar1=-0.5 * S, scalar2=1.5 * S,
                                    op0=OP.mult, op1=OP.add)
            nc.vector.tensor_mul(y[:T, :w], y[:T, :w], u[:T, :w])
            nm = pst.tile([128, NB], f32, tag=tag + "nm")
            nc.vector.scalar_tensor_tensor(out=nm[:T, :w],
                                           in0=mvall[:T, 0, o:o + w],
                                           scalar=-1.0, in1=y[:T, :w],
                                           op0=OP.mult, op1=OP.mult)
            return y, nm

        def ln_apply(x_sl, rs2, nm2, j, tag):
            xn = pxn.tile([128, D], bf16, tag=tag)
            nc.gpsimd.tensor_scalar(out=xn[:T], in0=x_sl,
                                    scalar1=rs2[:T, j:j + 1],
                                    scalar2=nm2[:T, j:j + 1],
                                    op0=OP.mult, op1=OP.add)
            ps = psq.tile([128, KC_D, T], bf16, tag="ps_big")
            for c in range(KC_D):
                nc.tensor.transpose(ps[:128, c, :T],
                                    xn[:T, ts(c, 128)], id_sb[:T, :T])
            return ps

        xf = x_d.flatten_outer_dims()
        of = out_d.flatten_outer_dims()

        # small groups at both ends: the ramp-in attention phases run
        # before any FFN work exists to pair with (PE exposed at the
        # throttled clock), and the final tail drains alone - keep those
        # exposed phases short, full-size groups in the steady state
        groups = []
        tok = 0
        for nb in [4, 6] + [NB] * 5 + [4]:
            groups.append((tok, nb))
            tok += nb * T
        assert sum(nb for _, nb in groups) == BC

        live = {}

        def emit_ln1(gi):
            tok0, nb = groups[gi]
            XnT = pxt.tile([128, KC_D, NTOK_MAX], bf16, tag="xnt")
            mvall = pst.tile([128, nc.vector.BN_AGGR_DIM, NB], f32, tag="xn1mv")
            xres = []
            for i in range(nb):
                xt_ = px.tile([128, D], f32, tag="xres")
                nc.sync.dma_start(out=xt_[:T],
                                  in_=xf[tok0 + i * T: tok0 + (i + 1) * T])
                xres.append(xt_)
            # finish/apply in small chunks (4+3+3 for full groups): each
            # apply burst only waits on its own chunk's DMAs+stats, and
            # the serial gpsimd/transpose bursts stay short so they never
            # dam the in-order PE queue for long
            cuts = [0, 4, 7, nb] if nb == NB else [0, (nb + 1) // 2, nb]
            ln_stats(xres[0][:T], mvall, 0)
            yield
            for ci in range(len(cuts) - 1):
                lo, hi = cuts[ci], cuts[ci + 1]
                for i in range(max(1, lo), hi):
                    ln_stats(xres[i][:T], mvall, i)
                y_nm = ln_finish(mvall, hi - lo, f"xn1{ci}", 1.0, o=lo)
                yield
                for i in range(lo, hi):
                    ps = ln_apply(xres[i][:T], *y_nm, i - lo, "xn1")
                    nc.scalar.activation(out=XnT[:, :, i * T:(i + 1) * T],
                                         in_=ps[:, :, :T], func=AF.Copy)
                    yield
            live[gi] = dict(xres=xres, XnT=XnT)
            yield

        def emit_qkv(gi):
            """Q/K/V projections for group gi. Generator (PE-dense filler)."""
            tok0, nb = groups[gi]
            ntok = nb * T
            XnT = live[gi]["XnT"]
            QT = pqk.tile([128, KC_D, NTOK_MAX], bf16, tag="qt")
            KT = pqk.tile([128, KC_D, NTOK_MAX], bf16, tag="kt")
            for di, (dst, w_sb, b_sb) in enumerate(((QT, wq_sb, bq_sb),
                                                    (KT, wk_sb, bk_sb))):
                slabs = [(h0, min(h0 + 500, ntok))
                         for h0 in range(0, ntok, 500)]
                for m in range(KC_D):
                    # both slabs' chains interleave across the two psum
                    # banks: each scalar copy gets a full chain of runway
                    grp = slabs[:2]
                    pss_ = [psq.tile([128, 512], f32, tag="ps_big",
                                     name=f"ps_qk{k}")
                            for k in range(len(grp))]
                    for kc in range(KC_D):
                        for k, (h0, h1) in enumerate(grp):
                            nc.tensor.matmul(pss_[k][:, :h1 - h0],
                                             w_sb[:, kc, ts(m, 128)],
                                             XnT[:, kc, h0:h1],
                                             start=(kc == 0),
                                             stop=(kc == KC_D - 1))
                    for k, (h0, h1) in enumerate(grp):
                        if b_sb is not None:
                            nc.vector.tensor_scalar_add(
                                out=dst[:, m, h0:h1], in0=pss_[k][:, :h1 - h0],
                                scalar1=b_sb[:, m:m + 1])
                        else:
                            nc.scalar.activation(out=dst[:, m, h0:h1],
                                                 in_=pss_[k][:, :h1 - h0],
                                                 func=AF.Copy)
                    yield
            V = pv.tile([128, NB, H, VW], bf16, tag="v")
            # ones columns carry 1/S_O so the softmax denominator comes out
            # pre-scaled: rden = S_O/den, making O fp8 at scale S_O free.
            nc.gpsimd.memset(V[:T, :nb, :, 64:VW], 1.0 / S_O)
            for b0 in range(0, nb, 2):
                pr = [b0] + ([b0 + 1] if b0 + 1 < nb else [])
                pss_ = [pstk.tile([128, D], f32, tag="ps_tok",
                                  name=f"ps_v{b}") for b in pr]
                for kc in range(KC_D):
                    for k, b in enumerate(pr):
                        nc.tensor.matmul(pss_[k][:T, :],
                                         XnT[:, kc, b * T:(b + 1) * T],
                                         wv_sb[:, kc, :],
                                         start=(kc == 0),
                                         stop=(kc == KC_D - 1))
                for k, b in enumerate(pr):
                    psh = pss_[k][:T].rearrange("p (h s) -> p h s", h=H)
                    if use_bv:
                        bvh = bv_sb[:T].rearrange("p (h s) -> p h s", h=H)
                        nc.vector.tensor_add(V[:T, b, :, 0:64], psh, bvh)
                    else:
                        nc.scalar.activation(out=V[:T, b, :, 0:64], in_=psh,
                                             func=AF.Copy)
                yield
            live[gi].update(QT=QT, KT=KT, V=V)
            yield

        def emit_attn(gi):
            """Attention for group gi -> per-batch O (bf16, [tok, chan])."""
            tok0, nb = groups[gi]
            QT, KT, V = live[gi]["QT"], live[gi]["KT"], live[gi]["V"]

            def front(b):
                bs = slice(b * T, (b + 1) * T)
                # emit head pairs (even, odd) interleaved: the two PE
                # row-groups (rows 0:64 / 64:128, via auto tile_position)
                # stream concurrently. Same-bank writers (h, h+2, h+4)
                # stay same-row-group, so they serialize - no concurrent
                # same-bank PSUM writes.
                # mask-add on the DVE: in the strict phasing the PE is the
                # critical engine and the attention phase runs throttled,
                # so +2 DVE ops/batch beat +2 PE preload matmuls/batch
                exs = []
                for g3 in range(2):
                    ps_s = psa.tile([128, 3, T], f32, tag="ps_att")
                    for j, h in enumerate((g3, g3 + 2, g3 + 4)):
                        mb, mo = h // 2, (h % 2) * 64
                        nc.tensor.matmul(ps_s[:T, j, :],
                                         KT[mo:mo + 64, mb, bs],
                                         QT[mo:mo + 64, mb, bs],
                                         start=True, stop=True)
                    sm = patt.tile([128, 3 * T], f32, tag="sm")
                    nc.vector.tensor_add(sm[:T], ps_s[:T, :, :], mask_sb[:T])
                    ex = patt.tile([128, 3 * T], bf16, tag="ex")
                    nc.scalar.activation(out=ex[:T], in_=sm[:T], func=AF.Exp)
                    exs.append(ex)
                return exs

            def back(b, exs):
                # both head-triples share one PSUM bank (2x3x66 fits), so
                # the normalization collapses to ONE reciprocal + ONE mul
                O_sb = po.tile([128, D], bf16, tag="o")
                rden = pst.tile([128, H], f32, tag="rden")
                ps_o = psa.tile([128, 2, 3, VW], f32, tag="ps_att")
                for g3 in range(2):
                    ex = exs[g3]
                    for j in range(3):
                        nc.tensor.matmul(ps_o[:T, g3, j, :],
                                         ex[:T, ts(j, T)],
                                         V[:T, b, g3 + 2 * j, :],
                                         start=True, stop=True)
                nc.vector.reciprocal(out=rden[:T, :], in_=ps_o[:T, :, :, 64:65])
                rsl = rden[:T].rearrange("p (g j) -> p j g", g=2)
                rb = bass.AP(tensor=rsl.tensor, offset=rsl.offset,
                             ap=[*map(list, rsl.ap), [0, 64]])
                osl = O_sb[:T].rearrange("p (c two s) -> p c two s",
                                         two=2, s=64)
                nc.vector.tensor_mul(
                    osl, ps_o[:T].rearrange("p g c v -> p c g v")[:, :, :, 0:64],
                    rb)
                return O_sb

            prev = None
            Os = []
            for b in range(nb):
                exs = front(b)
                if prev is not None:
                    Os.append(back(prev, prev_exs))
                prev, prev_exs = b, exs
                yield
            Os.append(back(prev, prev_exs))
            live[gi]["O"] = Os

        def emit_tail(gi):
            """O transpose, proj+residual, LN2, FFN, store for group gi."""
            tok0, nb = groups[gi]
            ntok = nb * T
            xres, Os = live[gi]["xres"], live[gi]["O"]
            OT = pot.tile([128, NB, KC_D, 128], f8, tag="ot")
            X2 = px2.tile([128, NB, D], f32, tag="x2")
            mvall = pst.tile([128, nc.vector.BN_AGGR_DIM, NB], f32, tag="xn2mv")
            Xn2T = pxt.tile([128, KC_D, NTOK_MAX], f8, tag="xn2t")
            DS_P = 1.0 / (S_O * S_W)    # proj PSUM descale
            DS_1 = S_H / (S_X2 * S_W)   # FFN1 PSUM descale * hidden scale
            DS_2 = 1.0 / (S_H * S_W)    # FFN2 PSUM descale

            def o_transpose(i):
                # deferred from the attention phase: runs in the dense
                # phase where the PE clock is warm and it has runway
                ps_t = psa.tile([128, KC_D, T], bf16, tag="ps_att")
                for c in range(KC_D):
                    nc.tensor.transpose(ps_t[:128, c, :],
                                        Os[i][:T, ts(c, 128)], id_sb[:T, :T])
                nc.scalar.activation(out=OT[:, i, :, 0:T], in_=ps_t[:, :, :],
                                     func=AF.Copy)

            def fin_app(i):
                # chunk A = batches 0-4 == FFN1's first token slab, so
                # slab-0 FFN1 matmuls can be pulled into the proj loop
                if i == min(4, nb - 1):
                    w = i + 1
                    y_nm = ln_finish(mvall, w, "xn2a", S_X2)
                    for k in range(w):
                        ps = ln_apply(X2[:T, k, :], y_nm[0], y_nm[1], k, "xn2")
                        nc.vector.tensor_copy(Xn2T[:, :, k * T:(k + 1) * T],
                                              ps[:, :, :T])
                elif nb == 10 and i == 7:
                    y_nm = ln_finish(mvall, 3, "xn2c", S_X2, o=5)
                    for k in range(5, 8):
                        ps = ln_apply(X2[:T, k, :], y_nm[0], y_nm[1], k - 5,
                                      "xn2")
                        nc.vector.tensor_copy(Xn2T[:, :, k * T:(k + 1) * T],
                                              ps[:, :, :T])
                elif i == nb - 1 and nb > 5:
                    o = 8 if nb == 10 else 5
                    y_nm = ln_finish(mvall, nb - o, "xn2b", S_X2, o=o)
                    for k in range(o, nb):
                        ps = ln_apply(X2[:T, k, :], y_nm[0], y_nm[1], k - o,
                                      "xn2")
                        nc.vector.tensor_copy(Xn2T[:, :, k * T:(k + 1) * T],
                                              ps[:, :, :T])

            def ffn1_slab(h0, h1, m):
                ps = psq.tile([128, 512], f32, tag="ps_big")
                nc.tensor.matmul(ps[:, :h1 - h0], w12_sb[:, m, :, :],
                                 Xn2T[:, 0:2, h0:h1],
                                 start=True, stop=False,
                                 perf_mode=PM.DoubleRow)
                nc.tensor.matmul(ps[:, :h1 - h0], w1k_sb[:, ts(m, 128)],
                                 Xn2T[:, 2, h0:h1],
                                 start=False, stop=True)
                b0 = h0 // T
                nbh = (h1 - h0) // T
                dsth = HT[:, m // 2, b0:b0 + nbh, m % 2, 0:T]
                psb = ps[:, 0:nbh * T].rearrange("p (b t) -> p b t", t=T)
                if use_b1:
                    nc.scalar.activation(out=dsth, in_=psb, func=AF.Relu,
                                         bias=b1_sb[:, m:m + 1], scale=DS_1)
                else:
                    nc.scalar.activation(out=dsth, in_=psb, func=AF.Relu,
                                         scale=DS_1)

            HT = phid.tile([128, NJ_H, NB, 2, 128], f8, tag="hid")
            o_transpose(0)
            if nb > 1:
                o_transpose(1)
            # proj pair-pipelined like FFN2: both psum banks' short
            # DR chains interleave so the DVE epilogue has runway
            for i0 in range(0, nb, 2):
                pr = [i0] + ([i0 + 1] if i0 + 1 < nb else [])
                for i in (i0 + 2, i0 + 3):
                    if i < nb:
                        o_transpose(i)
                pss_ = [pstk.tile([128, D], f32, tag="ps_tok",
                                  name=f"ps_pj_{i}") for i in pr]
                # stationary trimmed to T tokens: LDWEIGHTS cost is its
                # column count, so the 28 pad columns are pure overhead
                for k, i in enumerate(pr):
                    nc.tensor.matmul(pss_[k][0:T, :], OT[:, i, 0:2, 0:T],
                                     wp2_sb[:, :, :], start=True, stop=False,
                                     perf_mode=PM.DoubleRow)
                for k, i in enumerate(pr):
                    nc.tensor.matmul(pss_[k][0:T, :], OT[:, i, 2, 0:T],
                                     wpk_sb[:, :], start=False, stop=True)
                for k, i in enumerate(pr):
                    ps = pss_[k]
                    if use_bp:
                        nc.vector.scalar_tensor_tensor(out=ps[:T, :],
                                                       in0=ps[:T, :],
                                                       scalar=DS_P,
                                                       in1=bp_sb[:T, :],
                                                       op0=OP.mult, op1=OP.add)
                        nc.vector.tensor_add(X2[:T, i, :], ps[:T, :],
                                             xres[i][:T, :])
                    else:
                        nc.vector.scalar_tensor_tensor(out=X2[:T, i, :],
                                                       in0=ps[:T, :],
                                                       scalar=DS_P,
                                                       in1=xres[i][:T, :],
                                                       op0=OP.mult, op1=OP.add)
                    ln_stats(X2[:T, i, :], mvall, i)
                    fin_app(i)
                    # slab-0 FFN1 pulled forward: ready once chunk A is
                    # applied, it fills the PE while chunk B normalizes
                    if nb == 10 and i >= 6:
                        for m in range(3 * (i - 6), 3 * (i - 5)):
                            ffn1_slab(0, 500, m)
                    yield
            # FFN2 software-pipelined across batch pairs: the two psum
            # banks' accumulation chains interleave, so the DVE epilogue
            # of batch i has a full 12-matmul window before its bank is
            # reallocated (with per-batch chains the WAR hit every 6)
            def ffn2_pair(i0):
                pr = [i0] + ([i0 + 1] if i0 + 1 < nb else [])
                pss_ = [pstk.tile([128, D], f32, tag="ps_tok",
                                  name=f"ps_f2_{i}") for i in pr]
                for j in range(NJ_H):
                    for k in range(len(pr)):
                        nc.tensor.matmul(pss_[k][0:T, :],
                                         HT[:, j, pr[k], :, 0:T],
                                         w22_sb[:, j, :, :],
                                         start=(j == 0),
                                         stop=(j == NJ_H - 1),
                                         perf_mode=PM.DoubleRow)
                for k, i in enumerate(pr):
                    ps = pss_[k]
                    ot_ = pout.tile([128, D], f32, tag="outt")
                    if use_b2:
                        nc.vector.scalar_tensor_tensor(out=ps[:T, :],
                                                       in0=ps[:T, :],
                                                       scalar=DS_2,
                                                       in1=b2_sb[:T, :],
                                                       op0=OP.mult, op1=OP.add)
                        nc.vector.tensor_add(ot_[:T, :], ps[:T, :],
                                             X2[:T, i, :])
                    else:
                        nc.vector.scalar_tensor_tensor(out=ot_[:T, :],
                                                       in0=ps[:T, :],
                                                       scalar=DS_2,
                                                       in1=X2[:T, i, :],
                                                       op0=OP.mult, op1=OP.add)
                    nc.sync.dma_start(out=of[tok0 + i * T: tok0 + (i + 1) * T],
                                      in_=ot_[:T, :])

            # slab-0 batches' FFN2 only needs slab-0 HT (complete after
            # the proj loop) - interleave those pairs into the slab-1
            # FFN1 emission so the FFN boundary never drains the PE
            done2 = 0
            for m in range(KC_H):
                for h0 in range(500 if nb == 10 else 0, ntok, 500):
                    ffn1_slab(h0, min(h0 + 500, ntok), m)
                if nb == 10 and m in (3, 7):
                    ffn2_pair(2 * done2)
                    done2 += 1
                if m % 2 == 1:
                    yield
            for i0 in range(2 * done2, nb, 2):
                ffn2_pair(i0)
                yield
            del live[gi]

        def alternate(*its, weights=None):
            its = [it for it in its if it is not None]
            if weights is None:
                weights = [1] * len(its)
            w = {id(it): wt for it, wt in zip(its, weights)}
            while its:
                for it in list(its):
                    for _ in range(w.get(id(it), 1)):
                        try:
                            next(it)
                        except StopIteration:
                            its.remove(it)
                            break

        def drive(it, steps):
            # advance a generator a bounded number of yields; True if done
            for _ in range(steps):
                try:
                    next(it)
                except StopIteration:
                    return True
            return False

        # software pipeline, per iteration g:
        #   P1: [QKV(g) x LN1(g+1)]   - dense Q/K streams hide LN1(g+1)'s
        #       serial finish/apply chain, so qkv(g+1) never stalls on XnT
        #   P2: [tail(g-1) x attn(g)] - FFN streams hide attention's gaps,
        #       attention matmuls hide the LN2 chain inside tail
        # Strict phase separation, driven by PE clock throttling (HAM):
        # the attention phase's small matmuls can't keep the activity
        # window busy, so it always runs at the 1.2GHz throttled clock -
        # but it is LDWEIGHTS/latency-bound and barely cares. The dense
        # full-row streaming work (qkv + proj/FFN) DOES care, so it is
        # kept out of the attention windows entirely:
        #   PA: attn(g) [+ ln1(g+2) chain: DVE/gpsimd work, PE only sees
        #       its clock-insensitive transposes]
        #   PB: tail(g-1) + qkv(g+1), all dense, one long K=8/8 burst
        ng = len(groups)
        alternate(emit_ln1(0), emit_weights())
        alternate(emit_qkv(0), emit_ln1(1), weights=[2, 1])
        for g in range(ng - 1):
            alternate(emit_attn(g),
                      emit_ln1(g + 2) if g + 2 < ng else None)
            alternate(emit_tail(g - 1) if g >= 1 else None,
                      emit_qkv(g + 1))
        # drain: the last attention rides with the second-to-last tail
        # (nothing dense is left to protect from HAM), and the last tail
        # interleaves with it batch-by-batch
        alternate(emit_attn(ng - 1), emit_tail(ng - 2), weights=[1, 2])
        alternate(emit_tail(ng - 1))

    nc.compile()
    return nc


def _get_nc(*flags):
    if flags not in _NC_CACHE:
        _NC_CACHE[flags] = _build_nc(*flags)
    return _NC_CACHE[flags]


def _prep_inputs(x, wq, wk, wv, wproj, bproj, w1, b1, w2, b2, g1, beta1, g2, beta2):
    import ml_dtypes
    f = np.float32
    bf = ml_dtypes.bfloat16
    f8 = ml_dtypes.float8_e4m3
    # stack per-head projections into [D, D] with head h at columns h*HS:(h+1)*HS
    wq_f = np.ascontiguousarray(wq.transpose(1, 0, 2).reshape(D, D), dtype=f)
    wk_f = np.ascontiguousarray(wk.transpose(1, 0, 2).reshape(D, D), dtype=f)
    wv_f = np.ascontiguousarray(wv.transpose(1, 0, 2).reshape(D, D), dtype=f)
    scale = np.float32(HS ** 0.5)
    # fold LN1 affine into qkv weights, LN2 affine into w1
    wq_p = (g1[:, None] * wq_f) * scale
    wk_p = g1[:, None] * wk_f
    wv_p = g1[:, None] * wv_f
    w1_p = g2[:, None] * w1
    bq = (beta1 @ wq_f) * scale
    bk = beta1 @ wk_f
    bv = beta1 @ wv_f
    b1_p = b1 + beta2 @ w1
    bp = bproj
    b2_p = b2

    def lay(w, kc, dt, s=1.0):
        # [K, M] -> [128, kc, M] with K split into kc chunks of 128
        wl = np.asarray(w, dtype=f) * s
        return np.ascontiguousarray(
            wl.reshape(kc, 128, w.shape[1]).transpose(1, 0, 2)
        ).astype(dt)

    def q8(w):
        return np.clip(np.asarray(w, f) * S_W, -448.0, 448.0).astype(f8)

    def layb(bias, kc, s=1.0):
        return np.ascontiguousarray((bias * s).reshape(kc, 128).T, dtype=f)

    # transposed causal mask, tiled for 3 heads: keep (t >= u)
    maskT = np.full((T, T), MASK_VAL, dtype=f)
    maskT[np.triu_indices(T)] = 0.0
    mask3 = np.ascontiguousarray(np.tile(maskT, (1, 3))).astype(bf)

    w2_q = q8(w2)                     # [1536, 384]
    wp_q = q8(wproj)                  # [384, 384]
    w1_q = q8(w1_p)                   # [384, 1536]
    # DoubleRow planar-pair layouts (k-tile pair per partition)
    wp_dr = np.ascontiguousarray(
        wp_q.reshape(KC_D, 128, D)[0:2].transpose(1, 0, 2))      # [128,2,384]
    wp_k2 = np.ascontiguousarray(wp_q[256:384, :])               # [128,384]
    w1_dr = np.ascontiguousarray(
        w1_q.reshape(KC_D, 128, KC_H, 128)[0:2]
        .transpose(1, 2, 0, 3))                                  # [128,12,2,128]
    w1_k2 = np.ascontiguousarray(w1_q[256:384, :])               # [128,1536]
    w2_dr = np.ascontiguousarray(
        w2_q.reshape(NJ_H, 2, 128, D).transpose(2, 0, 1, 3))     # [128,6,2,384]

    shared = {
        "wq_l": lay(wq_p, KC_D, bf), "wk_l": lay(wk_p, KC_D, bf),
        "wv_l": lay(wv_p, KC_D, bf),
        "wp_dr": wp_dr, "wp_k2": wp_k2, "w1_dr": w1_dr, "w1_k2": w1_k2,
        "w2_dr": w2_dr,
        "mask3": mask3, "ident": np.eye(128, dtype=f).astype(bf),
    }
    use_bv = bool(np.any(bv))
    use_bp = bool(np.any(bp))
    use_b2 = bool(np.any(b2_p))
    use_bq = bool(np.any(bq)) or bool(np.any(bk))
    use_b1 = bool(np.any(b1_p))
    if use_bq:
        shared["bq_l"] = layb(bq, KC_D)
        shared["bk_l"] = layb(bk, KC_D)
    if use_b1:
        # the ACT applies relu(ps*DS_1 + bias): bias must carry S_H
        shared["b1_l"] = layb(b1_p, KC_H, S_H)
    if use_bv:
        shared["bv_bc"] = np.ascontiguousarray(np.tile(bv.astype(f), (128, 1)))
    if use_bp:
        shared["bp_bc"] = np.ascontiguousarray(np.tile(np.asarray(bp, f), (128, 1)))
    if use_b2:
        shared["b2_bc"] = np.ascontiguousarray(np.tile(np.asarray(b2_p, f), (128, 1)))
    return shared, (use_bv, use_bp, use_b2, use_bq, use_b1)


def kernel(**inputs):
    from concourse.bass_utils import run_bass_kernel_spmd

    x = np.asarray(inputs["x"], dtype=np.float32)
    shared, flags = _prep_inputs(
        x, *[np.asarray(inputs[k], dtype=np.float32) for k in
             ("wq", "wk", "wv", "wproj", "bproj", "w1", "b1", "w2", "b2",
              "g1", "beta1", "g2", "beta2")])
    nc = _get_nc(*flags)
    in_maps = []
    for c in range(N_CORES):
        m = dict(shared)
        m["x"] = np.ascontiguousarray(x[c * BC:(c + 1) * BC])
        in_maps.append(m)
    res = run_bass_kernel_spmd(nc, in_maps, core_ids=list(range(N_CORES)))
    return np.concatenate([res.results[i]["out"] for i in range(N_CORES)], axis=0)

